# revision 11
# baseline (speedup 1.0000x reference)
"""Trainium2 Bass kernel for LocalWindowAttention.

Reference semantics (per batch b):
    pad seq 4000 -> 4096, split into 32 windows of 128 tokens.
    qkv = x @ w_qkv.T + b_qkv ; per-window per-head softmax(q k^T / sqrt(64)) @ v
    out = o @ w_out.T + b_out ; drop padded tail.

Sharding: data-parallel over batch. Core b computes batch b fully.

Per-core design (v2 -- all matmul stages sized so the PE streams at one
column/cycle with full 128-row output tiles):
  - x staged feature-major xt[e, t] (e on partitions, 8 chunks of 128).
  - Q computed feature-major into plain [P, 8, CT] tiles.  K is evicted
    from its projection PSUM straight into a zero-padded per-head layout
    kz[P, H, CT] via two half-partition ACT copies (the complement halves
    are memset to zero once), so every score matmul is a full-128-row
    contraction based at partition 0:
        S^T[tk, tq] = sum_d K[d, tk] Q[d, tq]
    (base-partition-64 row-tiled matmuls crash the NEFF runtime -- probed;
    the zero-padding trick is the safe way to select one head per matmul.)
  - V computed token-major, stored per-head with a ones column appended:
        v'[tk, h, 0:64] = V_h, v'[tk, h, 64] = 1
    AV runs "O transposed" with E stationary:
        O_t[tq, 0:65] = sum_tk E[tk, tq] v'[tk, :]
    so column 64 delivers the softmax denominator for free -- no separate
    denominator matmuls, no cross-partition broadcast.  Normalisation is a
    per-partition tensor_scalar multiply by 1/D (denominators live on the
    tq partition like O_t does).
  - O_t is transposed back to feature-major via regular matmuls against a
    128x128 identity (NOT transpose_mode, which is ~5x slower), feeding the
    out-projection as its stationary operand.
  - Software pipeline: window w emits scores/AV(w), transposes(w-1) and
    out-projection(w-2), so every PE stage has slack for its cross-engine
    evict/normalise chain.
  - Everything on-chip is fp16 (same PE speed as bf16, 3 extra mantissa
    bits; |scores| <= ~3 so exp() stays well inside fp16 range, exp is
    computed without max-subtraction which is exact for softmax).  1/sqrt(64)
    is folded into w_q on the host.  Accumulation is always fp32 in PSUM.
"""

import sys
import numpy as np

for _p in ("/opt/trn_rl_repo", "/root/.axon_site/_ro/trn_rl_repo"):
    if _p not in sys.path:
        sys.path.append(_p)

P = 128          # partitions
E = 1024         # embed dim
H = 16           # heads
D = 64           # head dim
W = 128          # window
B = 8            # batch
S = 4000         # seq len
SP = 4096        # padded seq len
NW = SP // W     # 32 windows
CW = 4           # windows per chunk
CT = CW * W      # 512 tokens per chunk
EC = 8           # e-chunks of 128

F16 = np.float16

_cache = {}


def build_nc(n_chunks, s_out, has_bqk, has_bout):
    """Build + compile the single-core Bass program (same program for all cores)."""
    from concourse import bacc, tile, mybir

    dt = mybir.dt
    AF = mybir.ActivationFunctionType

    nc = bacc.Bacc(None, target_bir_lowering=False, debug=False)

    xt_d = nc.dram_tensor("xt", [n_chunks, P, EC, CT], dt.float16, kind="ExternalInput")
    wqkv_d = nc.dram_tensor("wqkv", [P, EC, 3 * E], dt.float16, kind="ExternalInput")
    wout_d = nc.dram_tensor("wout", [P, EC, E], dt.float16, kind="ExternalInput")
    id_d = nc.dram_tensor("ident", [P, P], dt.float16, kind="ExternalInput")
    out_d = nc.dram_tensor("out", [s_out, E], dt.float16, kind="ExternalOutput")
    if has_bqk:
        bqk_d = nc.dram_tensor("bqk", [P, 2, EC], dt.float32, kind="ExternalInput")
    if has_bout:
        cb_d = nc.dram_tensor("cb", [P, 2, 512], dt.float32, kind="ExternalInput")

    with tile.TileContext(nc) as tc:
        with (
            tc.tile_pool(name="const", bufs=1) as constp,
            tc.tile_pool(name="xp", bufs=2) as xp,
            tc.tile_pool(name="qp", bufs=2) as qp,
            tc.tile_pool(name="kzp", bufs=1) as kzp,
            tc.tile_pool(name="vp", bufs=1) as vp,
            tc.tile_pool(name="ep", bufs=2) as ep,
            tc.tile_pool(name="otp", bufs=3) as otp,
            tc.tile_pool(name="ofp", bufs=3) as ofp,
            tc.tile_pool(name="rp", bufs=4) as rp,
            tc.tile_pool(name="fpl", bufs=2) as fpl,
            # single PSUM pool: every tile here is <= 2KB = one bank, 8 bufs
            # cover all 8 banks; the 8-call reuse distance gives each stage's
            # cross-engine evict/normalize chain time to drain.
            tc.tile_pool(name="psA", bufs=8, space="PSUM") as psA,
        ):
            ident = constp.tile([P, P], dt.float16)
            nc.sync.dma_start(ident[:], id_d[:])
            # chunk 0's x arrives alongside the first weight slices
            xt_first = xp.tile([P, EC, CT], dt.float16, tag="xt", name="xt_first")
            nc.sync.dma_start(xt_first[:], xt_d[0])
            # weights split per (q|k|v, e-chunk) in exactly the order the
            # first chunk's projection consumes them, so the PE can start
            # as soon as the q slices land instead of waiting for all 8.4MB
            wq = constp.tile([P, EC, 3 * E], dt.float16)
            for part in range(3):
                for ec in range(EC):
                    nc.sync.dma_start(
                        wq[:, ec, part * E:(part + 1) * E],
                        wqkv_d[:, ec, part * E:(part + 1) * E],
                    )
            wo = constp.tile([P, EC, E], dt.float16)
            for ec in range(EC):
                nc.sync.dma_start(wo[:, ec, :], wout_d[:, ec, :])
            if has_bqk:
                bqk = constp.tile([P, 2, EC], dt.float32)
                nc.sync.dma_start(bqk[:], bqk_d[:])
            if has_bout:
                cb = constp.tile([P, 2, 512], dt.float32)
                nc.sync.dma_start(cb[:], cb_d[:])

            # V tiles are persistent so their ones-column (written once here)
            # survives: the per-chunk evict only writes columns 0:64.
            ones_c = constp.tile([P, CW, H, 1], dt.float16)
            nc.vector.memset(ones_c[:], 1.0)
            v_tiles = []
            for i in range(2):
                vt = vp.tile([P, CW, H, 65], dt.float16, tag=f"v{i}", name=f"v{i}")
                nc.vector.tensor_copy(vt[:, :, :, 64:65], ones_c[:])
                v_tiles.append(vt)
            # kz zero halves never change: clear the two persistent tiles once.
            kz_tiles = []
            for i in range(2):
                kzt = kzp.tile([P, H, CT], dt.float16, tag=f"kz{i}", name=f"kz{i}")
                nc.gpsimd.memset(kzt[:], 0.0)
                kz_tiles.append(kzt)

            def stage_scores(wi, kz_sb, q_sb, e_sb):
                """per-window scores + exp; kz's zero half selects head h
                from the q f-tile pair under a full-128 contraction."""
                ws = slice(wi * W, (wi + 1) * W)
                for p in range(EC):
                    ps_s = psA.tile([P, 2, W], dt.float32, tag="ps")
                    for hh in range(2):
                        nc.tensor.matmul(
                            ps_s[:, hh, :],
                            kz_sb[:, 2 * p + hh, ws],
                            q_sb[:, p, ws],
                            start=True, stop=True,
                        )
                    nc.scalar.activation(
                        e_sb[:, 2 * p:2 * p + 2, :], ps_s[:], AF.Exp,
                    )

            def stage_av_group(wi, g, e_sb, v_sb, o_t):
                """4 heads of AV (denominator rides along as column 64),
                then normalize into o_t."""
                ps_a = psA.tile([P, 4, W], dt.float32, tag="ps")
                for j in range(4):
                    h = 4 * g + j
                    nc.tensor.matmul(
                        ps_a[:, j, 0:65],
                        e_sb[:, h, :],
                        v_sb[:, wi, h, :],
                        start=True, stop=True,
                    )
                rdi = rp.tile([P, 4, 1], dt.float32, tag="rdi")
                nc.vector.reciprocal_approx_fast(rdi[:], ps_a[:, :, 64:65])
                for j in range(4):
                    h = 4 * g + j
                    nc.vector.tensor_scalar_mul(
                        o_t[:, h * D:(h + 1) * D], ps_a[:, j, 0:64], rdi[:, j, :],
                    )

            def stage_transpose(o_t, o_f, lo, hi):
                """feature-major O via regular matmul against identity."""
                for pp in range(lo, hi):
                    ps_t = psA.tile([P, W], dt.float32, tag="ps")
                    nc.tensor.matmul(
                        ps_t[:], o_t[:, pp * P:(pp + 1) * P], ident[:],
                        start=True, stop=True,
                    )
                    nc.vector.tensor_copy(o_f[:, pp, :], ps_t[:])

            def stage_outproj(o_f, row0, rows):
                f_sb = fpl.tile([P, 2, 512], dt.float16, tag="f")
                for fh in range(2):
                    ps_o = psA.tile([P, 512], dt.float32, tag="ps")
                    for cc in range(EC):
                        nc.tensor.matmul(
                            ps_o[:],
                            o_f[:, cc, :],
                            wo[:, cc, fh * 512:(fh + 1) * 512],
                            start=(cc == 0),
                            stop=(cc == EC - 1),
                        )
                    if has_bout:
                        nc.vector.tensor_add(f_sb[:, fh, :], ps_o[:], cb[:, fh, :])
                    else:
                        nc.scalar.activation(f_sb[:, fh, :], ps_o[:], AF.Copy)
                nc.sync.dma_start(out_d[row0:row0 + rows, :], f_sb[:rows])

            pend_t = None   # window awaiting transpose: (o_t, o_f, row0, rows)
            pend_o = None   # window awaiting out-projection: (o_f, row0, rows)
            for c in range(n_chunks):
                if c == 0:
                    xt = xt_first
                else:
                    xt = xp.tile([P, EC, CT], dt.float16, tag="xt")
                    nc.sync.dma_start(xt[:], xt_d[c])

                q_sb = qp.tile([P, EC, CT], dt.float16, tag="q")
                kz_sb = kz_tiles[c % 2]
                v_sb = v_tiles[c % 2]

                # ---- Q and K (feature-major): psum[f_tile, t].  K's evict
                # scatters the two heads of the f-tile into kz's zero-padded
                # per-head slots (partitions preserved, so head parity keeps
                # its natural base partition). ----
                for which in (0, 1):
                    for ft in range(EC):
                        ps = psA.tile([P, 512], dt.float32, tag="ps")
                        off = which * E + ft * P
                        for ec in range(EC):
                            nc.tensor.matmul(
                                ps[:],
                                wq[:, ec, off:off + P],
                                xt[:, ec, :],
                                start=(ec == 0),
                                stop=(ec == EC - 1),
                            )
                        if which == 0:
                            if has_bqk:
                                nc.scalar.activation(
                                    q_sb[:, ft, :], ps[:], AF.Identity,
                                    bias=bqk[:, 0, ft:ft + 1],
                                )
                            else:
                                nc.scalar.activation(q_sb[:, ft, :], ps[:], AF.Copy)
                        else:
                            for hh in range(2):
                                pr = slice(hh * D, hh * D + D)
                                if has_bqk:
                                    nc.scalar.activation(
                                        kz_sb[pr, 2 * ft + hh, :], ps[pr],
                                        AF.Identity, bias=bqk[pr, 1, ft:ft + 1],
                                    )
                                else:
                                    nc.scalar.activation(
                                        kz_sb[pr, 2 * ft + hh, :], ps[pr], AF.Copy,
                                    )

                # ---- V (token-major, per-head + ones col): psum[t, f] ----
                for wi in range(CW):
                    for fh in range(2):
                        ps = psA.tile([P, 8, 64], dt.float32, tag="ps")
                        off = 2 * E + fh * 512
                        for ec in range(EC):
                            nc.tensor.matmul(
                                ps[:],
                                xt[:, ec, wi * W:(wi + 1) * W],
                                wq[:, ec, off:off + 512],
                                start=(ec == 0),
                                stop=(ec == EC - 1),
                            )
                        nc.vector.tensor_copy(
                            v_sb[:, wi, fh * 8:(fh + 1) * 8, 0:64], ps[:],
                        )

                # ---- attention, 2-deep software pipeline:
                # slot w: scores/AV(w) | transposes(w-1) | out-proj(w-2)
                for wi in range(CW):
                    g = c * CW + wi
                    row0 = g * W
                    rows = min(s_out - row0, W)
                    if rows <= 0:
                        continue
                    e_sb = ep.tile([P, H, W], dt.float16, tag="e")
                    o_t = otp.tile([P, H * D], dt.float16, tag="ot")
                    stage_scores(wi, kz_sb, q_sb, e_sb)
                    # AV groups with w-1's transposes woven between them: the
                    # PSUM bank of AV group g is reused 8 psA calls later, by
                    # which time its normalize chain has drained.
                    for g in range(4):
                        stage_av_group(wi, g, e_sb, v_sb, o_t)
                        if pend_t is not None:
                            stage_transpose(pend_t[0], pend_t[1], 2 * g, 2 * g + 2)
                    if pend_o is not None:
                        stage_outproj(*pend_o)
                    o_f = ofp.tile([P, EC, W], dt.float16, tag="of")
                    pend_o = pend_t[1:] if pend_t is not None else None
                    pend_t = (o_t, o_f, row0, rows)

            # drain the pipeline
            if pend_o is not None:
                stage_outproj(*pend_o)
            if pend_t is not None:
                stage_transpose(pend_t[0], pend_t[1], 0, 8)
                stage_outproj(*pend_t[1:])

    nc.compile()
    return nc


def prep_inputs(x, w_qkv, b_qkv, w_out, b_out, n_chunks, s_out):
    """Host-side staging: pad, transpose, cast, fold scale into w_q."""
    sp = n_chunks * CT
    nb = x.shape[0]

    wqkvT = np.ascontiguousarray(w_qkv.T).astype(np.float32).copy()
    wqkvT[:, :E] *= 1.0 / np.sqrt(D)
    wqkv_sb = np.ascontiguousarray(
        wqkvT.reshape(EC, P, 3 * E).transpose(1, 0, 2)
    ).astype(F16)

    woutT = np.ascontiguousarray(w_out.T)
    wout_sb = np.ascontiguousarray(
        woutT.reshape(EC, P, E).transpose(1, 0, 2)
    ).astype(F16)

    base = {
        "wqkv": wqkv_sb,
        "wout": wout_sb,
        "ident": np.eye(P, dtype=F16),
    }

    has_bqk = bool(np.any(b_qkv[:2 * E]))
    has_bout = bool(np.any(b_out)) or bool(np.any(b_qkv[2 * E:]))
    if has_bqk:
        bqk = np.stack(
            [b_qkv[:E].reshape(EC, P).T / np.sqrt(D),
             b_qkv[E:2 * E].reshape(EC, P).T], axis=1
        ).astype(np.float32)  # (P, 2, EC)
        base["bqk"] = np.ascontiguousarray(bqk)
    if has_bout:
        cbv = (b_out + b_qkv[2 * E:] @ w_out.T).astype(np.float32)  # (E,)
        base["cb"] = np.ascontiguousarray(
            np.broadcast_to(cbv.reshape(1, 2, 512), (P, 2, 512))
        ).copy()

    in_maps = []
    for b in range(nb):
        xp_ = np.zeros((sp, E), dtype=np.float32)
        xp_[:min(s_out, x.shape[1])] = x[b][:s_out]
        xT = np.ascontiguousarray(xp_.T)  # (E, sp)
        xt_sb = np.ascontiguousarray(
            xT.reshape(EC, P, n_chunks, CT).transpose(2, 1, 0, 3)
        ).astype(F16)  # (n_chunks, P, EC, CT)
        m = dict(base)
        m["xt"] = xt_sb
        in_maps.append(m)
    return in_maps, has_bqk, has_bout


def run(x, w_qkv, b_qkv, w_out, b_out, n_chunks=NW // CW, s_out=S, trace=False):
    from concourse import bass_utils

    in_maps, has_bqk, has_bout = prep_inputs(
        x, w_qkv, b_qkv, w_out, b_out, n_chunks, s_out
    )
    key = (n_chunks, s_out, has_bqk, has_bout)
    if key not in _cache:
        _cache[key] = build_nc(*key)
    nc = _cache[key]

    res = bass_utils.run_bass_kernel_spmd(
        nc, in_maps, core_ids=list(range(len(in_maps))), trace=trace,
    )
    out = np.stack(
        [r["out"].astype(np.float32) for r in res.results], axis=0
    )
    return out, res


def kernel(x, w_qkv, b_qkv, w_out, b_out):
    x = np.asarray(x, dtype=np.float32)
    w_qkv = np.asarray(w_qkv, dtype=np.float32)
    b_qkv = np.asarray(b_qkv, dtype=np.float32)
    w_out = np.asarray(w_out, dtype=np.float32)
    b_out = np.asarray(b_out, dtype=np.float32)
    out, _ = run(x, w_qkv, b_qkv, w_out, b_out)
    return out
